# revision 11
# baseline (speedup 1.0000x reference)
"""LoRA linear layer on 8 Trainium2 NeuronCores.

Computes y = x @ W^T + b + 2.0 * (x @ A^T) @ B^T for
x:[4,4096,1024], W:[1024,1024], b:[1024], A:[16,1024], B:[1024,16].

Host side folds the LoRA update into the weight (W_eff = W + 2*B@A, an exact
algebraic identity), so the device kernel is a single GEMM + bias. Sharding is
data-parallel over the 16384 tokens: each of the 8 cores computes a
[2048, 1024] output slice with replicated weights.

Device kernel (per core): y_c[m,o] = sum_d xT_c[d,m] * WeffT[d,o] + b[o]
  - xT_c  [1024, 2048] f32 (host-transposed so the contraction dim d lands on
    SBUF partitions for both matmul operands)
  - WeffT [1024, 1024] f32, fully resident in SBUF
  - float32r matmuls (full PE rate at N=512), fp32 PSUM accumulation
  - bias broadcast to 128 partitions on host; fused add on the DVE during
    PSUM->SBUF eviction
"""

import numpy as np

import concourse.mybir as mybir
import concourse.tile as tile
from concourse import bacc
from concourse.bass_utils import run_bass_kernel_spmd

N_CORES = 8
P = 128
D = 1024  # in_features (contraction)
O = 1024  # out_features
M_TOTAL = 4 * 4096  # tokens
M = M_TOTAL // N_CORES  # tokens per core
KO = D // P  # k-subtiles
SC = 512  # m super-chunk (DMA granularity)
SCALING = 2.0

# Set by test harnesses to capture profiling info; harmless otherwise.
TRACE = False
LAST_RESULT = None

_NC_CACHE = None


def _build_nc():
    f32 = mybir.dt.float32
    f32r = mybir.dt.float32r

    nc = bacc.Bacc("TRN2", debug=False)
    xT = nc.dram_tensor("xT", [D, M], f32r, kind="ExternalInput")
    wT = nc.dram_tensor("wT", [D, O], f32r, kind="ExternalInput")
    bias = nc.dram_tensor("bias", [P, O], f32, kind="ExternalInput")
    y = nc.dram_tensor("y", [M, O], f32, kind="ExternalOutput")

    xT_v = xT[:].rearrange("(ko p) m -> p ko m", p=P)  # [128, 8, 2048]
    wT_v = wT[:].rearrange("(ko p) o -> p ko o", p=P)  # [128, 8, 1024]
    y_v = y[:].rearrange("(mt p) o -> p mt o", p=P)  # [128, 16, 1024]

    n_sc = M // SC
    with tile.TileContext(nc) as tc:
        with (
            tc.tile_pool(name="wpool", bufs=1) as wpool,
            tc.tile_pool(name="bpool", bufs=1) as bpool,
            tc.tile_pool(name="xpool", bufs=4) as xpool,
            tc.tile_pool(name="opool", bufs=6) as opool,
            tc.tile_pool(name="psum", bufs=8, space="PSUM") as psum,
        ):
            # x super-chunks arrive as two half-K tiles; W split per k-subtile.
            # Issue order interleaves them (x_h0, W0-3, x_h1, W4-7) so the
            # first matmuls only wait on ~1.5 MiB and the ramp paces the
            # W stream instead of idling behind it.
            xh = {}

            def load_x_half(sc, h):
                t = xpool.tile([P, KO // 2, SC], f32r, tag="xt")
                nc.sync.dma_start(
                    t[:],
                    xT_v[
                        :,
                        (KO // 2) * h : (KO // 2) * (h + 1),
                        sc * SC : (sc + 1) * SC,
                    ],
                )
                xh[(sc, h)] = t

            wt = [None] * KO

            def load_w(ko):
                t = wpool.tile([P, O], f32r, tag=f"w{ko}")
                nc.sync.dma_start(t[:], wT_v[:, ko, :])
                wt[ko] = t

            # Zero warmup tile: ~14 throwaway matmuls keep the PE busy while
            # the first x/W slices stream in, so the HAM clock-gate is warm
            # (2.4 GHz) by the time real matmuls start.
            zt = wpool.tile([P, 512], mybir.dt.bfloat16, tag="warm")
            nc.gpsimd.memset(zt[:], 0.0)
            wps = psum.tile([P, 512], mybir.dt.float32, tag="ps", name="wps")
            for _ in range(12):
                nc.tensor.matmul(wps[:], zt[:, :P], zt[:], start=True, stop=True)

            # sc0's x arrives per-ko (256 KiB) interleaved with W slices so the
            # first real matmul only waits on ~0.75 MiB.
            x0 = []
            for ko in range(KO):
                t = xpool.tile([P, SC], f32r, tag="x0", bufs=KO, name=f"x0_{ko}")
                nc.sync.dma_start(t[:], xT_v[:, ko, 0:SC])
                x0.append(t)
                load_w(ko)
            def x_slice(sc, ko, mt_i):
                if sc == 0:
                    return x0[ko][:, mt_i * P : (mt_i + 1) * P]
                return xh[(sc, ko // (KO // 2))][
                    :, ko % (KO // 2), mt_i * P : (mt_i + 1) * P
                ]

            def evict_half(ps, ot, half):
                nc.vector.tensor_tensor(
                    ot[:, half * 512 : (half + 1) * 512],
                    ps[:],
                    bt[:, half * 512 : (half + 1) * 512],
                    mybir.AluOpType.add,
                )

            MPC = SC // P  # m-tiles per super-chunk

            # Every super-chunk runs ko-outer: all four m-tiles accumulate
            # simultaneously across the 8 single-bank PSUM groups, so each W/x
            # slice is consumed as it lands during the ramp and the PE never
            # sits behind one large dependency. Evictions + stores are inlined
            # right behind each group's stop so PSUM slots recycle smoothly
            # into the next super-chunk.
            bt = None
            for sc in range(n_sc):
                if sc + 1 < n_sc:
                    load_x_half(sc + 1, 0)
                    load_x_half(sc + 1, 1)
                if sc == 0:
                    # bias is only needed at the first eviction; keep it behind
                    # the x stream the PE needs sooner
                    bt = bpool.tile([P, O], f32)
                    nc.sync.dma_start(bt[:], bias[:])
                pss = [
                    [
                        psum.tile(
                            [P, 512], mybir.dt.float32, tag="ps", name=f"ps{sc}_{i}_{h}"
                        )
                        for h in range(2)
                    ]
                    for i in range(MPC)
                ]
                ots = [
                    opool.tile([P, O], f32, tag="ot", name=f"ot{sc}_{i}")
                    for i in range(MPC)
                ]
                for ko in range(KO):
                    last = ko == KO - 1
                    for mt_i in range(MPC):
                        mt = sc * MPC + mt_i
                        for half in range(2):
                            nc.tensor.matmul(
                                pss[mt_i][half][:],
                                x_slice(sc, ko, mt_i),
                                wt[ko][:, half * 512 : (half + 1) * 512],
                                start=ko == 0,
                                stop=last,
                            )
                        if last:
                            for half in range(2):
                                evict_half(pss[mt_i][half], ots[mt_i], half)
                                nc.gpsimd.dma_start(
                                    y_v[:, mt, half * 512 : (half + 1) * 512],
                                    ots[mt_i][:, half * 512 : (half + 1) * 512],
                                )

    nc.compile()
    return nc


def _get_nc():
    global _NC_CACHE
    if _NC_CACHE is None:
        _NC_CACHE = _build_nc()
    return _NC_CACHE


def kernel(x, W, b, A, B):
    global LAST_RESULT
    x = np.ascontiguousarray(np.asarray(x, dtype=np.float32))
    W = np.asarray(W, dtype=np.float32)
    b = np.asarray(b, dtype=np.float32)
    A = np.asarray(A, dtype=np.float32)
    B = np.asarray(B, dtype=np.float32)
    assert x.shape == (4, 4096, D) and W.shape == (O, D)
    assert b.shape == (O,) and A.shape[1] == D and B.shape[0] == O

    # Fold the LoRA update into the weight: x@W^T + s*(x@A^T)@B^T = x@(W + s*B@A)^T
    Weff = (
        W.astype(np.float64) + SCALING * (B.astype(np.float64) @ A.astype(np.float64))
    ).astype(np.float32)
    WeffT = np.ascontiguousarray(Weff.T)  # [D, O]
    bias_rep = np.ascontiguousarray(np.broadcast_to(b[None, :], (P, O)))

    xr = x.reshape(M_TOTAL, D)
    in_maps = []
    for c in range(N_CORES):
        xTc = np.ascontiguousarray(xr[c * M : (c + 1) * M].T)  # [D, M]
        in_maps.append({"xT": xTc, "wT": WeffT, "bias": bias_rep})

    nc = _get_nc()
    res = run_bass_kernel_spmd(
        nc, in_maps, core_ids=list(range(N_CORES)), trace=TRACE
    )
    LAST_RESULT = res

    out = np.concatenate([res.results[c]["y"] for c in range(N_CORES)], axis=0)
    return out.reshape(x.shape[0], x.shape[1], O)


# revision 12
# speedup vs baseline: 1.0856x; 1.0856x over previous
"""LoRA linear layer on 8 Trainium2 NeuronCores.

Computes y = x @ W^T + b + 2.0 * (x @ A^T) @ B^T for
x:[4,4096,1024], W:[1024,1024], b:[1024], A:[16,1024], B:[1024,16].

Host side folds the LoRA update into the weight (W_eff = W + 2*B@A, an exact
algebraic identity), so the device kernel is a single GEMM + bias. Sharding is
data-parallel over the 16384 tokens: each of the 8 cores computes a
[2048, 1024] output slice with replicated weights.

Device kernel (per core): y_c[m,o] = sum_d xT_c[d,m] * WeffT[d,o] + b[o]
  - xT_c  [1024, 2048] f32 (host-transposed so the contraction dim d lands on
    SBUF partitions for both matmul operands)
  - WeffT [1024, 1024] f32, fully resident in SBUF
  - float32r matmuls (full PE rate at N=512), fp32 PSUM accumulation
  - bias broadcast to 128 partitions on host; fused add on the DVE during
    PSUM->SBUF eviction
"""

import numpy as np

import concourse.mybir as mybir
import concourse.tile as tile
from concourse import bacc
from concourse.bass_utils import run_bass_kernel_spmd

N_CORES = 8
P = 128
D = 1024  # in_features (contraction)
O = 1024  # out_features
M_TOTAL = 4 * 4096  # tokens
M = M_TOTAL // N_CORES  # tokens per core
KO = D // P  # k-subtiles
SC = 512  # m super-chunk (DMA granularity)
SCALING = 2.0

# Set by test harnesses to capture profiling info; harmless otherwise.
TRACE = False
LAST_RESULT = None

_NC_CACHE = None


def _build_nc():
    f32 = mybir.dt.float32
    f32r = mybir.dt.float32r

    nc = bacc.Bacc("TRN2", debug=False)
    xT = nc.dram_tensor("xT", [D, M], f32r, kind="ExternalInput")
    wT = nc.dram_tensor("wT", [D, O], f32r, kind="ExternalInput")
    bias = nc.dram_tensor("bias", [P, O], f32, kind="ExternalInput")
    y = nc.dram_tensor("y", [M, O], f32, kind="ExternalOutput")

    xT_v = xT[:].rearrange("(ko p) m -> p ko m", p=P)  # [128, 8, 2048]
    wT_v = wT[:].rearrange("(ko p) o -> p ko o", p=P)  # [128, 8, 1024]
    y_v = y[:].rearrange("(mt p) o -> p mt o", p=P)  # [128, 16, 1024]

    n_sc = M // SC
    with tile.TileContext(nc) as tc:
        with (
            tc.tile_pool(name="wpool", bufs=1) as wpool,
            tc.tile_pool(name="bpool", bufs=1) as bpool,
            tc.tile_pool(name="xpool", bufs=16) as xpool,
            tc.tile_pool(name="opool", bufs=6) as opool,
            tc.tile_pool(name="psum", bufs=8, space="PSUM") as psum,
        ):
            # x arrives in 256 KiB per-(super-chunk, ko) granules and W in
            # 512 KiB per-ko granules, so every matmul group only ever waits on
            # a small slice and DMA catch-up shows up as many short PE gaps
            # (which keep the HAM clock-gate warm) instead of multi-us stalls.
            xts = {}

            def load_x(sc):
                for ko in range(KO):
                    t = xpool.tile([P, SC], f32r, tag="xt", name=f"x{sc}_{ko}")
                    nc.sync.dma_start(t[:], xT_v[:, ko, sc * SC : (sc + 1) * SC])
                    xts[(sc, ko)] = t

            wt = [None] * KO

            def load_w(ko):
                t = wpool.tile([P, O], f32r, tag=f"w{ko}")
                nc.sync.dma_start(t[:], wT_v[:, ko, :])
                wt[ko] = t

            # Zero warmup tile: ~14 throwaway matmuls keep the PE busy while
            # the first x/W slices stream in, so the HAM clock-gate is warm
            # (2.4 GHz) by the time real matmuls start.
            zt = wpool.tile([P, 512], mybir.dt.bfloat16, tag="warm")
            nc.gpsimd.memset(zt[:], 0.0)
            wps = psum.tile([P, 512], mybir.dt.float32, tag="ps", name="wps")
            for _ in range(12):
                nc.tensor.matmul(wps[:], zt[:, :P], zt[:], start=True, stop=True)

            # sc0's x interleaved with W slices so the first real matmul only
            # waits on ~0.75 MiB; bias right behind (it gates all evictions).
            for ko in range(KO):
                t = xpool.tile([P, SC], f32r, tag="xt", name=f"x0_{ko}")
                nc.sync.dma_start(t[:], xT_v[:, ko, 0:SC])
                xts[(0, ko)] = t
                load_w(ko)
            bt = bpool.tile([P, O], f32)
            nc.sync.dma_start(bt[:], bias[:])

            def x_slice(sc, ko, mt_i):
                return xts[(sc, ko)][:, mt_i * P : (mt_i + 1) * P]

            def evict_half(ps, ot, half):
                nc.vector.tensor_tensor(
                    ot[:, half * 512 : (half + 1) * 512],
                    ps[:],
                    bt[:, half * 512 : (half + 1) * 512],
                    mybir.AluOpType.add,
                )

            MPC = SC // P  # m-tiles per super-chunk

            # Every super-chunk runs ko-outer: all four m-tiles accumulate
            # simultaneously across the 8 single-bank PSUM groups, so each W/x
            # slice is consumed as it lands during the ramp and the PE never
            # sits behind one large dependency. Evictions + stores are inlined
            # right behind each group's stop so PSUM slots recycle smoothly
            # into the next super-chunk.
            for sc in range(n_sc):
                if sc + 1 < n_sc:
                    load_x(sc + 1)
                pss = [
                    [
                        psum.tile(
                            [P, 512], mybir.dt.float32, tag="ps", name=f"ps{sc}_{i}_{h}"
                        )
                        for h in range(2)
                    ]
                    for i in range(MPC)
                ]
                ots = [
                    opool.tile([P, O], f32, tag="ot", name=f"ot{sc}_{i}")
                    for i in range(MPC)
                ]
                for ko in range(KO):
                    last = ko == KO - 1
                    for mt_i in range(MPC):
                        mt = sc * MPC + mt_i
                        for half in range(2):
                            nc.tensor.matmul(
                                pss[mt_i][half][:],
                                x_slice(sc, ko, mt_i),
                                wt[ko][:, half * 512 : (half + 1) * 512],
                                start=ko == 0,
                                stop=last,
                            )
                        if last:
                            for half in range(2):
                                evict_half(pss[mt_i][half], ots[mt_i], half)
                                nc.gpsimd.dma_start(
                                    y_v[:, mt, half * 512 : (half + 1) * 512],
                                    ots[mt_i][:, half * 512 : (half + 1) * 512],
                                )

    nc.compile()
    return nc


def _get_nc():
    global _NC_CACHE
    if _NC_CACHE is None:
        _NC_CACHE = _build_nc()
    return _NC_CACHE


def kernel(x, W, b, A, B):
    global LAST_RESULT
    x = np.ascontiguousarray(np.asarray(x, dtype=np.float32))
    W = np.asarray(W, dtype=np.float32)
    b = np.asarray(b, dtype=np.float32)
    A = np.asarray(A, dtype=np.float32)
    B = np.asarray(B, dtype=np.float32)
    assert x.shape == (4, 4096, D) and W.shape == (O, D)
    assert b.shape == (O,) and A.shape[1] == D and B.shape[0] == O

    # Fold the LoRA update into the weight: x@W^T + s*(x@A^T)@B^T = x@(W + s*B@A)^T
    Weff = (
        W.astype(np.float64) + SCALING * (B.astype(np.float64) @ A.astype(np.float64))
    ).astype(np.float32)
    WeffT = np.ascontiguousarray(Weff.T)  # [D, O]
    bias_rep = np.ascontiguousarray(np.broadcast_to(b[None, :], (P, O)))

    xr = x.reshape(M_TOTAL, D)
    in_maps = []
    for c in range(N_CORES):
        xTc = np.ascontiguousarray(xr[c * M : (c + 1) * M].T)  # [D, M]
        in_maps.append({"xT": xTc, "wT": WeffT, "bias": bias_rep})

    nc = _get_nc()
    res = run_bass_kernel_spmd(
        nc, in_maps, core_ids=list(range(N_CORES)), trace=TRACE
    )
    LAST_RESULT = res

    out = np.concatenate([res.results[c]["y"] for c in range(N_CORES)], axis=0)
    return out.reshape(x.shape[0], x.shape[1], O)


# revision 13
# speedup vs baseline: 1.1206x; 1.0322x over previous
"""LoRA linear layer on 8 Trainium2 NeuronCores.

Computes y = x @ W^T + b + 2.0 * (x @ A^T) @ B^T for
x:[4,4096,1024], W:[1024,1024], b:[1024], A:[16,1024], B:[1024,16].

Host side folds the LoRA update into the weight (W_eff = W + 2*B@A, an exact
algebraic identity), so the device kernel is a single GEMM + bias. Sharding is
data-parallel over the 16384 tokens: each of the 8 cores computes a
[2048, 1024] output slice with replicated weights.

Device kernel (per core): y_c[m,o] = sum_d xT_c[d,m] * WeffT[d,o] + b[o]
  - xT_c  [1024, 2048] f32 (host-transposed so the contraction dim d lands on
    SBUF partitions for both matmul operands)
  - WeffT [1024, 1024] f32, fully resident in SBUF
  - float32r matmuls (full PE rate at N=512), fp32 PSUM accumulation
  - bias broadcast to 128 partitions on host; fused add on the DVE during
    PSUM->SBUF eviction
"""

import numpy as np

import concourse.mybir as mybir
import concourse.tile as tile
from concourse import bacc
from concourse.bass_utils import run_bass_kernel_spmd

N_CORES = 8
P = 128
D = 1024  # in_features (contraction)
O = 1024  # out_features
M_TOTAL = 4 * 4096  # tokens
M = M_TOTAL // N_CORES  # tokens per core
KO = D // P  # k-subtiles
SC = 512  # m super-chunk (DMA granularity)
SCALING = 2.0

# Set by test harnesses to capture profiling info; harmless otherwise.
TRACE = False
LAST_RESULT = None

_NC_CACHE = None


def _build_nc():
    f32 = mybir.dt.float32
    f32r = mybir.dt.float32r

    nc = bacc.Bacc("TRN2", debug=False)
    xT = nc.dram_tensor("xT", [D, M], f32r, kind="ExternalInput")
    wT = nc.dram_tensor("wT", [D, O], f32r, kind="ExternalInput")
    bias = nc.dram_tensor("bias", [P, O], f32, kind="ExternalInput")
    y = nc.dram_tensor("y", [M, O], f32, kind="ExternalOutput")

    xT_v = xT[:].rearrange("(ko p) m -> p ko m", p=P)  # [128, 8, 2048]
    wT_v = wT[:].rearrange("(ko p) o -> p ko o", p=P)  # [128, 8, 1024]
    y_v = y[:].rearrange("(mt p) o -> p mt o", p=P)  # [128, 16, 1024]

    n_sc = M // SC
    with tile.TileContext(nc) as tc:
        with (
            tc.tile_pool(name="wpool", bufs=1) as wpool,
            tc.tile_pool(name="bpool", bufs=1) as bpool,
            tc.tile_pool(name="xpool", bufs=16) as xpool,
            tc.tile_pool(name="opool", bufs=6) as opool,
            tc.tile_pool(name="psum", bufs=8, space="PSUM") as psum,
        ):
            # x arrives in 256 KiB per-(super-chunk, ko) granules and W in
            # 512 KiB per-ko granules, so every matmul group only ever waits on
            # a small slice and DMA catch-up shows up as many short PE gaps
            # (which keep the HAM clock-gate warm) instead of multi-us stalls.
            xts = {}

            def load_x(sc):
                for ko in range(KO):
                    t = xpool.tile([P, SC], f32r, tag="xt", name=f"x{sc}_{ko}")
                    nc.sync.dma_start(t[:], xT_v[:, ko, sc * SC : (sc + 1) * SC])
                    xts[(sc, ko)] = t

            wt = [None] * KO

            def load_w(ko):
                t = wpool.tile([P, O], f32r, tag=f"w{ko}")
                nc.sync.dma_start(t[:], wT_v[:, ko, :])
                wt[ko] = t

            # Zero warmup tile: ~14 throwaway matmuls keep the PE busy while
            # the first x/W slices stream in, so the HAM clock-gate is warm
            # (2.4 GHz) by the time real matmuls start.
            zt = wpool.tile([P, 512], mybir.dt.bfloat16, tag="warm")
            nc.gpsimd.memset(zt[:], 0.0)
            wps = psum.tile([P, 512], mybir.dt.float32, tag="ps", name="wps")
            for _ in range(12):
                nc.tensor.matmul(wps[:], zt[:, :P], zt[:], start=True, stop=True)

            # sc0's x interleaved with W slices so the first real matmul only
            # waits on ~0.75 MiB; bias right behind (it gates all evictions).
            for ko in range(KO):
                t = xpool.tile([P, SC], f32r, tag="xt", name=f"x0_{ko}")
                nc.sync.dma_start(t[:], xT_v[:, ko, 0:SC])
                xts[(0, ko)] = t
                load_w(ko)
            bt = bpool.tile([P, O], f32)
            nc.sync.dma_start(bt[:], bias[:])

            def x_slice(sc, ko, mt_i):
                return xts[(sc, ko)][:, mt_i * P : (mt_i + 1) * P]

            def evict_half(ps, ot, half):
                nc.vector.tensor_tensor(
                    ot[:, half * 512 : (half + 1) * 512],
                    ps[:],
                    bt[:, half * 512 : (half + 1) * 512],
                    mybir.AluOpType.add,
                )

            MPC = SC // P  # m-tiles per super-chunk

            # Every super-chunk runs ko-outer: all four m-tiles accumulate
            # simultaneously across the 8 single-bank PSUM groups, so each W/x
            # slice is consumed as it lands during the ramp and the PE never
            # sits behind one large dependency. Evictions + stores are inlined
            # right behind each group's stop so PSUM slots recycle smoothly
            # into the next super-chunk.
            for sc in range(n_sc - 1):
                if sc + 1 < n_sc:
                    load_x(sc + 1)
                pss = [
                    [
                        psum.tile(
                            [P, 512], mybir.dt.float32, tag="ps", name=f"ps{sc}_{i}_{h}"
                        )
                        for h in range(2)
                    ]
                    for i in range(MPC)
                ]
                ots = [
                    opool.tile([P, O], f32, tag="ot", name=f"ot{sc}_{i}")
                    for i in range(MPC)
                ]
                for ko in range(KO):
                    last = ko == KO - 1
                    for mt_i in range(MPC):
                        mt = sc * MPC + mt_i
                        for half in range(2):
                            nc.tensor.matmul(
                                pss[mt_i][half][:],
                                x_slice(sc, ko, mt_i),
                                wt[ko][:, half * 512 : (half + 1) * 512],
                                start=ko == 0,
                                stop=last,
                            )
                        if last:
                            for half in range(2):
                                evict_half(pss[mt_i][half], ots[mt_i], half)
                                nc.gpsimd.dma_start(
                                    y_v[:, mt, half * 512 : (half + 1) * 512],
                                    ots[mt_i][:, half * 512 : (half + 1) * 512],
                                )

            # Last super-chunk: mt-outer, so evictions and stores spread across
            # its whole span instead of piling up after the final matmul; the
            # very last m-tile runs its two output halves back to back so
            # half 0's eviction/store hides under half 1's matmuls.
            sc = n_sc - 1
            for mt_i in range(MPC):
                mt = sc * MPC + mt_i
                ot = opool.tile([P, O], f32, tag="ot", name=f"otf{mt_i}")
                final = mt_i == MPC - 1
                if not final:
                    ph = [
                        psum.tile([P, 512], mybir.dt.float32, tag="ps", name=f"pl{h}")
                        for h in range(2)
                    ]
                    for ko in range(KO):
                        for half in range(2):
                            nc.tensor.matmul(
                                ph[half][:],
                                x_slice(sc, ko, mt_i),
                                wt[ko][:, half * 512 : (half + 1) * 512],
                                start=ko == 0,
                                stop=ko == KO - 1,
                            )
                    for half in range(2):
                        evict_half(ph[half], ot, half)
                        nc.gpsimd.dma_start(
                            y_v[:, mt, half * 512 : (half + 1) * 512],
                            ot[:, half * 512 : (half + 1) * 512],
                        )
                else:
                    for half in range(2):
                        ps = psum.tile([P, 512], mybir.dt.float32, tag="ps", name="pf")
                        for ko in range(KO):
                            nc.tensor.matmul(
                                ps[:],
                                x_slice(sc, ko, mt_i),
                                wt[ko][:, half * 512 : (half + 1) * 512],
                                start=ko == 0,
                                stop=ko == KO - 1,
                            )
                        evict_half(ps, ot, half)
                        nc.sync.dma_start(
                            y_v[:, mt, half * 512 : (half + 1) * 512],
                            ot[:, half * 512 : (half + 1) * 512],
                        )

    nc.compile()
    return nc


def _get_nc():
    global _NC_CACHE
    if _NC_CACHE is None:
        _NC_CACHE = _build_nc()
    return _NC_CACHE


def kernel(x, W, b, A, B):
    global LAST_RESULT
    x = np.ascontiguousarray(np.asarray(x, dtype=np.float32))
    W = np.asarray(W, dtype=np.float32)
    b = np.asarray(b, dtype=np.float32)
    A = np.asarray(A, dtype=np.float32)
    B = np.asarray(B, dtype=np.float32)
    assert x.shape == (4, 4096, D) and W.shape == (O, D)
    assert b.shape == (O,) and A.shape[1] == D and B.shape[0] == O

    # Fold the LoRA update into the weight: x@W^T + s*(x@A^T)@B^T = x@(W + s*B@A)^T
    Weff = (
        W.astype(np.float64) + SCALING * (B.astype(np.float64) @ A.astype(np.float64))
    ).astype(np.float32)
    WeffT = np.ascontiguousarray(Weff.T)  # [D, O]
    bias_rep = np.ascontiguousarray(np.broadcast_to(b[None, :], (P, O)))

    xr = x.reshape(M_TOTAL, D)
    in_maps = []
    for c in range(N_CORES):
        xTc = np.ascontiguousarray(xr[c * M : (c + 1) * M].T)  # [D, M]
        in_maps.append({"xT": xTc, "wT": WeffT, "bias": bias_rep})

    nc = _get_nc()
    res = run_bass_kernel_spmd(
        nc, in_maps, core_ids=list(range(N_CORES)), trace=TRACE
    )
    LAST_RESULT = res

    out = np.concatenate([res.results[c]["y"] for c in range(N_CORES)], axis=0)
    return out.reshape(x.shape[0], x.shape[1], O)
